# revision 33
# baseline (speedup 1.0000x reference)
"""NonLocalFusionBlock Trainium2 kernel.

Data-parallel over batch: 8 batches -> 8 NeuronCores, one batch per core.

Per-core dataflow (all big matmuls in bf16, accumulation fp32 in PSUM):
  X2 [256, 4096]  (features_2d[b] flattened over H*W)
  X3 [512, 4096]  (features_3d[b] flattened over T*H3*W3)
  q  [c=128, n=4096]  = Wq @ X2 + bq        (c on partitions)
  kt [c=128, m=4096]  = Wk @ X3 + bk
  vr [m=4096, c=128]  = X3^T @ Wv^T         (m on partitions, 32 chunks)
  attT[m, n] = kt_chunk^T-contract: att chunk = lhsT(kt[:,mi]) . rhs(q)  -> PSUM
  et = exp(attT)  (no max subtraction: logits are O(10); scalar engine)
  acc[p, n] += et  (DVE partial sums over m-chunks)
  z[c, n] += vr[mi]^T @ et  (PSUM accumulation over 32 chunks)
  s[1, n] = ones^T @ acc; r = 1/s; R = ones x r (rank-1 matmul broadcast)
  zs = z * R                       -> softmax-normalized attention output^T
  out2[o, n] = Wo @ zs; x = X2 + out2 + bo'  (bo' = bo + Wo@bv, host folded)
  y = conv3x3(x, Wps) + bps  (as 9-tap matmul over zero-padded x in SBUF)

Self-contained: hardcodes shapes; no reads of /root/problem/*.
"""

import numpy as np
import ml_dtypes
from contextlib import ExitStack

import concourse.bass as bass
import concourse.mybir as mybir
import concourse.tile as tile
from concourse import bacc
from concourse.bass import ts, ds
from concourse.bass_utils import run_bass_kernel_spmd

P = 128
B = 8
C2D = 256
C3D = 512
NC = 128
H = W = 64
N2 = H * W            # 4096 query tokens
N3 = 4096             # key/value tokens (16*16*16)
NTW = 1024            # n-tile width for attention phase
HP = H + 2            # padded spatial (66)
F32 = mybir.dt.float32
BF16 = mybir.dt.bfloat16
EXP = mybir.ActivationFunctionType.Exp
COPY = mybir.ActivationFunctionType.Copy

_CACHED_NC = None


def _build():
    nc = bacc.Bacc(None)

    # inputs pre-tiled on host into 512-column blocks, contiguous per block
    x2_d = nc.declare_dram_parameter("x2", [8, C2D, 512], BF16, isOutput=False)
    x3_d = nc.declare_dram_parameter("x3", [8, C3D, 512], BF16, isOutput=False)
    wqT_d = nc.declare_dram_parameter("wqT", [C2D, NC], BF16, isOutput=False)
    wkT_d = nc.declare_dram_parameter("wkT", [C3D, NC], BF16, isOutput=False)
    wvT_d = nc.declare_dram_parameter("wvT", [C3D, NC], BF16, isOutput=False)
    woT_d = nc.declare_dram_parameter("woT", [NC, C2D], BF16, isOutput=False)
    wps_d = nc.declare_dram_parameter("wps", [C2D, 9 * C2D], BF16, isOutput=False)
    bq_d = nc.declare_dram_parameter("bq", [NC, 1], F32, isOutput=False)
    bk_d = nc.declare_dram_parameter("bk", [NC, 1], F32, isOutput=False)
    boe_d = nc.declare_dram_parameter("boe", [C2D, 1], F32, isOutput=False)
    bps_d = nc.declare_dram_parameter("bps", [C2D, 1], F32, isOutput=False)
    y_d = nc.declare_dram_parameter("y", [C2D, N2], F32, isOutput=True)

    with ExitStack() as ctx:
        tc = ctx.enter_context(tile.TileContext(nc))
        const = ctx.enter_context(tc.tile_pool(name="const", bufs=1))

        # ---- resident SBUF tensors (DMA issue order = consumption order;
        # inputs split into 512-column tiles so projections start as soon
        # as the first slices land) ----
        # Single DMA queue, strict critical-first order: everything the
        # first attention steps need (wq/bq, x2c0/1, wk/wv/bk, x3c0) lands
        # before bulk transfers start competing for bandwidth.
        wq_t = const.tile([P, 2, NC], BF16, tag="wq")
        nc.sync.dma_start(wq_t, wqT_d.rearrange("(kc p) m -> p kc m", p=P))
        bq_t = const.tile([P, 1], F32, tag="bq")
        nc.sync.dma_start(bq_t, bq_d[:, :])
        x2_c = [const.tile([P, 2, 512], BF16, tag=f"x2c{ct}", name=f"x2c{ct}")
                for ct in range(8)]
        x3_c = [const.tile([P, 4, 512], BF16, tag=f"x3c{ct}", name=f"x3c{ct}")
                for ct in range(8)]
        nc.sync.dma_start(x2_c[0], x2_d[0].rearrange("(kc p) n -> p kc n", p=P))
        nc.sync.dma_start(x2_c[1], x2_d[1].rearrange("(kc p) n -> p kc n", p=P))
        wk_t = const.tile([P, 4, NC], BF16, tag="wk")
        nc.sync.dma_start(wk_t, wkT_d.rearrange("(kc p) m -> p kc m", p=P))
        wv_t = const.tile([P, 4, NC], BF16, tag="wv")
        nc.sync.dma_start(wv_t, wvT_d.rearrange("(kc p) m -> p kc m", p=P))
        bk_t = const.tile([P, 1], F32, tag="bk")
        nc.sync.dma_start(bk_t, bk_d[:, :])
        nc.sync.dma_start(x3_c[0], x3_d[0].rearrange("(kc p) n -> p kc n", p=P))
        for ct in range(1, 8):
            nc.sync.dma_start(x3_c[ct],
                              x3_d[ct].rearrange("(kc p) n -> p kc n", p=P))
            if ct < 6:
                nc.sync.dma_start(x2_c[ct + 1],
                                  x2_d[ct + 1].rearrange("(kc p) n -> p kc n",
                                                         p=P))
        nc.sync.dma_start(x2_c[7], x2_d[7].rearrange("(kc p) n -> p kc n", p=P))

        wo_t = const.tile([P, C2D], BF16, tag="wo")
        nc.sync.dma_start(wo_t, woT_d[:, :])
        boe_t = const.tile([P, 2], F32, tag="boe")
        nc.sync.dma_start(boe_t, boe_d.rearrange("(c p) one -> p (c one)", p=P))
        wps_t = const.tile([P, 2, 9 * C2D], BF16, tag="wps")
        nc.sync.dma_start(wps_t, wps_d.rearrange("(kc p) f -> p kc f", p=P))
        bps_t = const.tile([P, 2], F32, tag="bps")
        nc.sync.dma_start(bps_t, bps_d.rearrange("(c p) one -> p (c one)", p=P))

        # all-ones [128,128]: S = ones @ acc computes column sums broadcast
        # to every partition in a single matmul
        ones_mat = const.tile([P, P], BF16, tag="ones_mat")
        nc.vector.memset(ones_mat, 1.0)

        q_c = [const.tile([P, NTW], BF16, tag=f"q{i}", name=f"q{i}")
               for i in range(4)]
        kt_g = [const.tile([P, 512], BF16, tag=f"kt{g}", name=f"kt{g}")
                for g in range(8)]
        vr_g = [const.tile([P, 4, NC], BF16, tag=f"vr{g}", name=f"vr{g}")
                for g in range(8)]
        xpad = const.tile([P, 2, HP * HP], BF16, tag="xpad")
        nc.gpsimd.memset(xpad, 0.0)

        # ---- phases 2+3: attention with conv matmuls interleaved ----
        with (
            tc.tile_pool(name="att_ps", bufs=2, space="PSUM") as patt,   # 2x2 banks
            tc.tile_pool(name="z_ps", bufs=1, space="PSUM") as pz,       # 2 banks
            tc.tile_pool(name="o2_ps", bufs=1, space="PSUM") as po2,     # 1 bank
            tc.tile_pool(name="conv_ps", bufs=1, space="PSUM") as pc,    # 1 bank
            tc.tile_pool(name="work", bufs=3) as pw,
            tc.tile_pool(name="accp", bufs=2) as pacc,
            tc.tile_pool(name="yp", bufs=3) as py,
        ):
            xpad_v = [xpad[:, c].rearrange("p (r w) -> p r w", r=HP) for c in (0, 1)]

            # QKV projection thunks (dripped into the first att tile's loop,
            # through the conv PSUM slot which is idle during nt0)
            def q_thunk(t):
                def run():
                    ps = pc.tile([P, 512], F32, tag="c", name="qkv_ps")
                    for kc in range(2):
                        nc.tensor.matmul(ps, wq_t[:, kc], x2_c[t][:, kc, :],
                                         start=(kc == 0), stop=(kc == 1))
                    nc.vector.tensor_scalar_add(q_c[t // 2][:, ts(t % 2, 512)],
                                                ps, bq_t)
                return run

            def k_thunk(g):
                def run():
                    ps = pc.tile([P, 512], F32, tag="c", name="qkv_ps")
                    for kc in range(4):
                        nc.tensor.matmul(ps, wk_t[:, kc], x3_c[g][:, kc, :],
                                         start=(kc == 0), stop=(kc == 3))
                    nc.vector.tensor_scalar_add(kt_g[g], ps, bk_t)
                return run

            def v_thunk(g):
                def run():
                    ps = pc.tile([P, 4, NC], F32, tag="c", name="qkv_ps")
                    for j in range(4):
                        for kc in range(4):
                            nc.tensor.matmul(ps[:, j],
                                             x3_c[g][:, kc, ts(j, NC)],
                                             wv_t[:, kc],
                                             start=(kc == 0), stop=(kc == 3))
                    nc.vector.tensor_copy(vr_g[g], ps)
                return run

            def conv_tile_thunks(j, oc, psum=None):
                """3x3 conv output rows 8j..8j+7, channels oc*128..: a list of
                thunks (18 matmuls + evacuate/DMA) to interleave into the
                attention loop wherever PE has slack."""
                pool, ptag, pwidth = psum or (pc, "c", 512)
                thunks = []
                state = {}

                def mk_mm(tap, kc, k):
                    def run():
                        if k == 0:
                            state["ps"] = pool.tile(
                                [P, pwidth], F32, tag=ptag,
                                name="conv_acc")[:, :512]
                        ps = state["ps"]
                        dy, dx = tap // 3, tap % 3
                        rhs = xpad_v[kc][:, j * 8 + dy:j * 8 + dy + 8, dx:dx + W]
                        nc.tensor.matmul(
                            ps.rearrange("p (r w) -> p r w", w=W),
                            wps_t[:, kc, ds(tap * C2D + oc * NC, NC)],
                            rhs, start=(k == 0), stop=(k == 17))
                    return run

                k = 0
                for tap in range(9):
                    for kc in range(2):
                        thunks.append(mk_mm(tap, kc, k))
                        k += 1

                def finish():
                    ps = state["ps"]
                    yt = py.tile([P, 512], F32, tag="y")
                    nc.vector.tensor_scalar_add(yt, ps, bps_t[:, oc:oc + 1])
                    nc.sync.dma_start(y_d[ds(oc * P, P), ts(j, 512)], yt)
                thunks.append(finish)
                return thunks

            # conv row-group j is runnable once xpad rows j*8..j*8+9 exist,
            # i.e. after nt > (8j+9)/16
            conv_ready = {1: [0], 2: [1, 2], 3: [3, 4], 4: [5, 6, 7]}

            def tail_thunks(nt, z, acc):
                """Normalization + out2 + residual for n-tile nt; dripped into
                the NEXT n-tile's loop so they never block the PE queue head."""
                thunks = []
                state = {}

                def mk_h(h):
                    col = nt * NTW + h * 512
                    r0 = col // W  # 8 rows of 64

                    def s_and_zs():
                        S_ps = patt.tile([P, NTW], F32, tag="att", name="S_ps")
                        nc.tensor.matmul(S_ps[:, :512], ones_mat,
                                         acc[:, ts(h, 512)], start=True, stop=True)
                        R = pw.tile([P, 512], F32, tag="R", name="R")
                        nc.vector.reciprocal_approx_fast(R, S_ps[:, :512])
                        zs = pw.tile([P, 512], BF16, tag="zs", name="zs")
                        nc.vector.tensor_mul(zs, z[:, ts(h, 512)], R)
                        state[h] = zs

                    def mk_oc(oc):
                        def out2():
                            zs = state[h]
                            o2 = po2.tile([P, 512], F32, tag="o2", name="o2")
                            nc.tensor.matmul(o2, wo_t[:, ts(oc, NC)], zs,
                                             start=True, stop=True)
                            dst = xpad_v[oc][:, 1 + r0:1 + r0 + 8, 1:1 + W]
                            nc.vector.scalar_tensor_tensor(
                                dst,
                                o2.rearrange("p (r w) -> p r w", w=W),
                                boe_t[:, oc:oc + 1],
                                x2_c[col // 512][:, oc, :].rearrange(
                                    "p (r w) -> p r w", w=W),
                                op0=mybir.AluOpType.add,
                                op1=mybir.AluOpType.add,
                            )
                        return out2

                    return [s_and_zs, mk_oc(0), mk_oc(1)]

                thunks.extend(mk_h(0))
                thunks.extend(mk_h(1))
                return thunks

            # bootstrap: just enough projections for nt0's first att steps
            q_thunk(0)()
            q_thunk(1)()
            k_thunk(0)()
            qkv_drip = [v_thunk(0)]
            for g in range(1, 8):
                qkv_drip.append(k_thunk(g))
                qkv_drip.append(v_thunk(g))
                if g in (3, 5):
                    qkv_drip.append(q_thunk(g - 1))  # q2, q4 -> q_c[1], q_c[2]
                if g in (4, 6):
                    qkv_drip.append(q_thunk(g - 1))  # q3, q5
            late_q = [q_thunk(6), q_thunk(7)]

            # flat software pipeline over 128 global steps: the z matmuls run
            # one step behind the att/exp stream, and each n-tile's first att
            # matmuls are emitted BEFORE the previous tile's closing z
            # matmuls so the exp stream never waits at tile boundaries.
            z = {}
            acc = {}
            pending = []
            prev_et = None  # (gs, et tile)
            for gs in range(129):
                nt, mi = divmod(gs, 32)
                if gs < 128:
                    if mi == 0:
                        acc[nt] = pacc.tile([P, NTW], BF16, tag="acc",
                                            name="acc")
                        extra = []
                        if nt == 0:
                            extra += qkv_drip
                        elif nt == 1:
                            extra += late_q
                        for j in conv_ready.get(nt, []):
                            for oc in range(2):
                                extra += conv_tile_thunks(j, oc)
                        pending = pending + extra
                    aps = patt.tile([P, NTW], F32, tag="att")
                    for h in range(2):
                        nc.tensor.matmul(aps[:, ts(h, 512)],
                                         kt_g[mi // 4][:, ts(mi % 4, NC)],
                                         q_c[nt][:, ts(h, 512)],
                                         start=True, stop=True)
                    et = pw.tile([P, NTW], BF16, tag="et")
                    nc.scalar.activation(et, aps, EXP)
                    if mi == 0:
                        nc.vector.tensor_copy(acc[nt], et)
                    else:
                        nc.vector.tensor_add(acc[nt], acc[nt], et)
                else:
                    et = None
                if prev_et is not None:
                    pgs, pet = prev_et
                    pnt, pmi = divmod(pgs, 32)
                    if pmi == 0:
                        z[pnt] = pz.tile([P, NTW], F32, tag="z", name="z")
                    for h in range(2):
                        nc.tensor.matmul(z[pnt][:, ts(h, 512)],
                                         vr_g[pmi // 4][:, pmi % 4],
                                         pet[:, ts(h, 512)],
                                         start=(pmi == 0), stop=(pmi == 31))
                    if pmi == 31:
                        # normalization/out2 thunks go to the FRONT so conv
                        # thunks that read their xpad writes stay ordered
                        pending = tail_thunks(pnt, z[pnt], acc[pnt]) + pending
                prev_et = (gs, et)
                # drip-feed tail + qkv + conv matmuls into PE slack
                if gs < 128:
                    left = 24 - mi if mi < 24 else 32 - mi
                    nrun = -(-len(pending) // max(1, left)) if pending else 0
                    for _ in range(nrun):
                        pending.pop(0)()
            deferred = pending

            # final n-tile's tail interleaved with the conv remnant (row
            # groups 5..7). Row-dependency order: h0 residual (rows 48-55)
            # before j=5; h1 (rows 56-63) before j=6,7.
            rot = [(pc, "c", 512), (po2, "o2", 512), (patt, "att", NTW),
                   (patt, "att", NTW)]
            for t in deferred[0:3]:
                t()
            j5 = (conv_tile_thunks(5, 0, rot[0]) + conv_tile_thunks(5, 1, rot[2]))
            for i, t in enumerate(j5):
                t()
                if i % 12 == 5 and deferred[3:]:
                    deferred.pop(3)()
            for t in deferred[3:]:
                t()
            ri = 1
            for j in (6, 7):
                for oc in range(2):
                    for t in conv_tile_thunks(j, oc, rot[ri % 4]):
                        t()
                    ri += 1

    nc.finalize()
    return nc


def _get_nc():
    global _CACHED_NC
    if _CACHED_NC is None:
        _CACHED_NC = _build()
    return _CACHED_NC


def _prep_host(features_2d, features_3d, Wq, bq, Wk, bk, Wv, bv, Wo, bo, Wps, bps):
    bf = ml_dtypes.bfloat16
    f2 = np.ascontiguousarray(
        features_2d.reshape(B, C2D, 8, 512).transpose(0, 2, 1, 3)).astype(bf)
    f3 = np.ascontiguousarray(
        features_3d.reshape(B, C3D, 8, 512).transpose(0, 2, 1, 3)).astype(bf)
    shared = {
        "wqT": np.ascontiguousarray(Wq.T).astype(bf),
        "wkT": np.ascontiguousarray(Wk.T).astype(bf),
        "wvT": np.ascontiguousarray(Wv.T).astype(bf),
        "woT": np.ascontiguousarray(Wo.T).astype(bf),
        "wps": np.ascontiguousarray(
            Wps.transpose(1, 2, 3, 0).reshape(C2D, 9 * C2D)).astype(bf),
        "bq": bq.reshape(NC, 1).astype(np.float32),
        "bk": bk.reshape(NC, 1).astype(np.float32),
        "boe": (bo + Wo @ bv).reshape(C2D, 1).astype(np.float32),
        "bps": bps.reshape(C2D, 1).astype(np.float32),
    }
    in_maps = []
    for b in range(B):
        m = dict(shared)
        m["x2"] = f2[b]
        m["x3"] = f3[b]
        in_maps.append(m)
    return in_maps


def kernel(**inputs):
    inputs = {k: np.asarray(v) for k, v in inputs.items()}
    in_maps = _prep_host(**inputs)
    nc = _get_nc()
    res = run_bass_kernel_spmd(nc, in_maps, core_ids=list(range(B)))
    y = np.stack([np.asarray(r["y"]) for r in res.results])
    return y.reshape(B, C2D, H, W).astype(np.float32)


if __name__ == "__main__":
    rng = np.random.default_rng(0)
    inp = {
        "features_2d": rng.standard_normal((B, C2D, H, W), dtype=np.float32),
        "features_3d": rng.standard_normal((B, C3D, 16, 16, 16), dtype=np.float32),
        "Wq": (rng.standard_normal((NC, C2D)) * 0.02).astype(np.float32),
        "bq": (rng.standard_normal((NC,)) * 0.02).astype(np.float32),
        "Wk": (rng.standard_normal((NC, C3D)) * 0.02).astype(np.float32),
        "bk": (rng.standard_normal((NC,)) * 0.02).astype(np.float32),
        "Wv": (rng.standard_normal((NC, C3D)) * 0.02).astype(np.float32),
        "bv": (rng.standard_normal((NC,)) * 0.02).astype(np.float32),
        "Wo": (rng.standard_normal((C2D, NC)) * 0.02).astype(np.float32),
        "bo": (rng.standard_normal((C2D,)) * 0.02).astype(np.float32),
        "Wps": (rng.standard_normal((C2D, C2D, 3, 3)) * 0.02).astype(np.float32),
        "bps": (rng.standard_normal((C2D,)) * 0.02).astype(np.float32),
    }
    out = kernel(**inp)
    print("kernel output", out.shape, out.dtype, float(np.abs(out).max()))
